# revision 1
# baseline (speedup 1.0000x reference)
"""Trainium2 Bass kernel for nn_EwaldPotential.

Math (per graph, all-real formulation of the complex reference):
  p[i,k]   = (r[i,:]/box) @ kvecs[k,:]            (phase in cycles)
  C = cos(2*pi*p), S = sin(2*pi*p)
  KRT[d,k] = sum_i k[i,d] C[i,k]   KIT = sum_i k[i,d] S[i,k]   (k_pot)
  VRT, VIT likewise from v
  A = q @ KRT, B = q @ KIT         logits L = C*A - S*B
  P = softmax_k(L)                 out = (P*C) @ VR - (P*S) @ VI

Sharding: data-parallel over the 8 graphs -> 1 graph per NeuronCore.

Key device tricks (all probed on HW):
 - phase via split matmuls: bf16(f_hi) @ kint plus fp16(f_lo) @ kint; kvecs
   are small integers so they are exact in bf16. A +M/-M (M = 1.5*2^23)
   row pair inside the bf16 matmul rounds the running sum to the nearest
   integer, and -f_hi/-f_lo rows re-subtract, leaving -frac(p) in PSUM
   (sin is 2*pi-periodic so the integer part is discardable). The ACT Sin
   table is only accurate on [-pi, pi]; this lands args exactly there.
 - C path likewise with a +0.25/-0.25 cycle shift (cos x = sin(x + pi/2)).
 - fp32 matmuls for k_pot and logits (TRN2 fp32 matmul error 1.5e-5 rel
   is fine there; it is NOT fine for the phase, hence the split trick).
 - post-softmax tensors (G = P*C, H = P*S, v_pot) go to fp16: DMA-xbar
   transposes (2-byte only) move K onto partitions for the final matmul.

This file is self-contained (shapes hardcoded; BIR wait-splitting patch
inlined: this container's walrus rejects >1 sem-wait per instruction).
"""
import json
import numpy as np
import ml_dtypes

import concourse.bass as bass
import concourse.tile as tile
from concourse import mybir
from concourse.bass_utils import run_bass_kernel_spmd

# ---------------------------------------------------------------- constants
B = 8
N_NODE = 2048
D = 64
DL = 4.0
TWOPI = 2.0 * np.pi
K_SQ_MAX = (TWOPI / DL) ** 2
MAGIC = 12582912.0          # 1.5 * 2^23: fp32 add rounds to integer
KP = 304                    # padded k-point count (297 valid)
KT = 384                    # transpose-padded width (3 x 128)
NCHUNK = N_NODE // 128

FP32 = mybir.dt.float32
BF16 = mybir.dt.bfloat16
FP16 = mybir.dt.float16

# ------------------------------------------------- walrus wait-split patch
_ws_counter = [0]


def _split_waits_json(bir_bytes: bytes) -> bytes:
    d = json.loads(bir_bytes)
    changed = False
    for fn in d.get("functions", []):
        for blk in fn.get("blocks", []):
            out = []
            for ins in blk.get("instructions", []):
                si = ins.get("sync_info")
                ow = (si or {}).get("on_wait") or []
                if len(ow) > 1:
                    changed = True
                    import os as _os
                    if _os.environ.get("BIRPATCH_DEBUG"):
                        print(f"SPLIT {ins.get('opcode')} {ins.get('name')} eng={ins.get('engine')} q={ins.get('queue')} nw={len(ow)}", flush=True)
                    for w in ow[:-1]:
                        _ws_counter[0] += 1
                        out.append({
                            "debug": ins.get("debug", 0),
                            "engine": ins.get("engine"),
                            "ins": [], "outs": [],
                            "name": f"I-wsplit{_ws_counter[0]}",
                            "opcode": "NoOp",
                            "sync_info": {"on_wait": [w], "on_update": []},
                        })
                    si["on_wait"] = [ow[-1]]
                out.append(ins)
            blk["instructions"] = out
    return json.dumps(d).encode() if changed else bir_bytes


def _install_bir_patch():
    import concourse.bass_utils as bu
    import concourse.bass2jax as b2j

    if getattr(b2j.compile_bir_kernel, "_wait_split", False):
        return
    orig = bu.compile_bir_kernel

    def wrapped(bir_json, tmpdir, neff_name="file.neff"):
        return orig(_split_waits_json(bir_json), tmpdir, neff_name)

    wrapped._wait_split = True
    b2j.compile_bir_kernel = wrapped
    bu.compile_bir_kernel = wrapped


# ------------------------------------------------------------ device kernel
def _build_nc(cfg=None):
    cfg = {**{'work_bufs': 3, 'gh_bufs': 3, 'ght_bufs': 3, 'ph_bufs': 2,
              'ab_bufs': 2, 'out_loop': True, 'p_on_dve': True, 'final_late': False,
              'tr_mode': 'pe', 'g_on_dve': True, 'ght_copy': 'act'}, **(cfg or {})}
    nc = bass.Bass("TRN2")
    lhsb = nc.dram_tensor("lhsb", [128, N_NODE], BF16, kind="ExternalInput")
    lhsh = nc.dram_tensor("lhsh", [128, N_NODE], FP16, kind="ExternalInput")
    rhsb = nc.dram_tensor("rhsb", [128, KP], BF16, kind="ExternalInput")
    rhsh = nc.dram_tensor("rhsh", [128, KP], FP16, kind="ExternalInput")
    kvcat = nc.dram_tensor("kvcat", [128, NCHUNK * 128], FP32, kind="ExternalInput")
    qt2 = nc.dram_tensor("qt2", [128, N_NODE], FP32, kind="ExternalInput")
    out_t = nc.dram_tensor("out_t", [64, N_NODE], FP32, kind="ExternalOutput")

    Sin = mybir.ActivationFunctionType.Sin
    Exp = mybir.ActivationFunctionType.Exp
    Copy = mybir.ActivationFunctionType.Copy
    mult = mybir.AluOpType.mult
    sub = mybir.AluOpType.subtract

    with tile.TileContext(nc) as tc:
        with tc.tile_pool(name="consts", bufs=1) as consts, \
             tc.tile_pool(name="work", bufs=cfg["work_bufs"]) as work, \
             tc.tile_pool(name="gh", bufs=cfg["gh_bufs"]) as gh, \
             tc.tile_pool(name="ghT", bufs=cfg["ght_bufs"]) as ghT, \
             tc.tile_pool(name="small", bufs=4) as small:

            t_lhsb = consts.tile([128, N_NODE], BF16, tag="lhsb")
            t_lhsh = consts.tile([128, N_NODE], FP16, tag="lhsh")
            t_rhsb = consts.tile([128, KP], BF16, tag="rhsb")
            t_rhsh = consts.tile([128, KP], FP16, tag="rhsh")
            t_kv = consts.tile([128, NCHUNK * 128], FP32, tag="kv")
            t_qt2 = consts.tile([128, N_NODE], FP32, tag="qt2")
            nc.sync.dma_start(t_lhsb, lhsb[:])
            nc.sync.dma_start(t_lhsh, lhsh[:])
            nc.sync.dma_start(t_rhsb, rhsb[:])
            nc.sync.dma_start(t_rhsh, rhsh[:])
            nc.sync.dma_start(t_kv, kvcat[:])
            nc.sync.dma_start(t_qt2, qt2[:])
            kvr = t_kv[:].rearrange("p (c d) -> p c d", c=NCHUNK)

            t_C = [consts.tile([128, KT], FP32, tag=f"C{c}", name=f"Ct{c}") for c in range(NCHUNK)]
            t_S = [consts.tile([128, KT], FP32, tag=f"S{c}", name=f"St{c}") for c in range(NCHUNK)]

            # ---------------- phase 1: trig + k_pot/v_pot accumulation
            with tc.tile_pool(name="ps_ph", bufs=cfg["ph_bufs"], space="PSUM") as ps_ph, \
                 tc.tile_pool(name="ps_kc", bufs=1, space="PSUM") as ps_kc:
                psKC = ps_kc.tile([128, KP], FP32, tag="KC")
                psKS = ps_kc.tile([128, KP], FP32, tag="KS")
                for c in range(NCHUNK):
                    sl = slice(c * 128, (c + 1) * 128)
                    pS = ps_ph.tile([128, KP], FP32, tag="pS")
                    pC = ps_ph.tile([128, KP], FP32, tag="pC")
                    # S path: [f_hi(3), +M, -M, -f_hi(3)] bf16, then -f_lo fp16
                    nc.tensor.matmul(pS, t_lhsb[0:8, sl], t_rhsb[0:8, :],
                                     start=True, stop=False, tile_position=(0, 0))
                    # C path: [f_hi, +0.25, +M, -M, -f_hi, -0.25] then -f_lo
                    nc.tensor.matmul(pC, t_lhsb[32:42, sl], t_rhsb[32:42, :],
                                     start=True, stop=False, tile_position=(32, 0))
                    nc.tensor.matmul(pS, t_lhsh[0:3, sl], t_rhsh[0:3, :],
                                     start=False, stop=True, tile_position=(0, 0))
                    nc.tensor.matmul(pC, t_lhsh[32:35, sl], t_rhsh[32:35, :],
                                     start=False, stop=True, tile_position=(32, 0))
                    # psum holds -frac -> sin(-2*pi*psum) = sin(2*pi*p)
                    nc.scalar.activation(t_S[c][:, 0:KP], pS, Sin, scale=-TWOPI)
                    nc.scalar.activation(t_C[c][:, 0:KP], pC, Sin, scale=-TWOPI)
                    nc.vector.memset(t_S[c][:, 297:KT], 0.0)
                    nc.vector.memset(t_C[c][:, 297:KT], 0.0)
                    # k_pot/v_pot: [KRT|VRT] += kv_chunk.T @ C, [KIT|VIT] += .. @ S
                    nc.tensor.matmul(psKC, kvr[:, c, :], t_C[c][:, 0:KP],
                                     start=(c == 0), stop=(c == NCHUNK - 1))
                    nc.tensor.matmul(psKS, kvr[:, c, :], t_S[c][:, 0:KP],
                                     start=(c == 0), stop=(c == NCHUNK - 1))

                # KRIT: rows 0:64 = KRT, rows 64:128 = KIT. PSUM can't be
                # DMA'd; stage KIT through SBUF for the partition move.
                t_KRIT = consts.tile([128, KP], FP32, tag="KRIT")
                t_kitst = consts.tile([64, KP], FP32, tag="kitst")
                nc.scalar.activation(t_KRIT[0:64, :], psKC[0:64, :], Copy)
                nc.scalar.activation(t_kitst[0:64, :], psKS[0:64, :], Copy)
                nc.sync.dma_start(t_KRIT[64:128, :], t_kitst[0:64, :])
                # v_pot to fp16 (rows 64:128 hold data, same partitions as psum)
                t_VRT = consts.tile([128, KT], FP16, tag="VRT")
                t_VIn = consts.tile([128, KT], FP16, tag="VIn")
                nc.vector.memset(t_VRT[64:128, KP:KT], 0.0)
                nc.vector.memset(t_VIn[64:128, KP:KT], 0.0)
                nc.scalar.activation(t_VRT[64:128, 0:KP], psKC[64:128, :], Copy)
                nc.scalar.activation(t_VIn[64:128, 0:KP], psKS[64:128, :], Copy,
                                     scale=-1.0)

            # transpose v_pot to [K, d] layout: VR_j, VIn_j fp16 [128, 64]
            t_VR = [consts.tile([128, 64], FP16, tag=f"VR{j}", name=f"VRt{j}") for j in range(3)]
            t_VI = [consts.tile([128, 64], FP16, tag=f"VI{j}", name=f"VIt{j}") for j in range(3)]
            for j in range(3):
                jsl = slice(j * 128, (j + 1) * 128)
                nc.sync.dma_start_transpose(t_VR[j], t_VRT[64:128, jsl])
                nc.sync.dma_start_transpose(t_VI[j], t_VIn[64:128, jsl])

            ps_ab = tc.alloc_tile_pool(name="ps_ab", bufs=cfg["ab_bufs"], space="PSUM")
            ps_out = tc.alloc_tile_pool(name="ps_out",
                                        bufs=(2 if (cfg["out_loop"] or cfg["final_late"]) else 1),
                                        space="PSUM")
            psO = [None] * 4
            if not (cfg["out_loop"] or cfg["final_late"]):
                psO = [ps_out.tile([64, 512], FP32, tag=f"O{g}", name=f"Ot{g}") for g in range(4)]

            GTs, HTs = [], []
            t_ident = None
            ps_ght = None
            if cfg["tr_mode"] in ("hybrid", "pe"):
                from concourse.masks import make_identity
                t_ident = consts.tile([128, 128], FP16, tag="ident")
                make_identity(nc, t_ident)
                ps_ght = tc.alloc_tile_pool(name="ps_ght", bufs=2, space="PSUM")
            # ---------------- phase 2: logits, softmax, inverse transform
            for c in range(NCHUNK):
                sl = slice(c * 128, (c + 1) * 128)
                if (cfg["out_loop"] or cfg["final_late"]) and c % 4 == 0:
                    psO[c // 4] = ps_out.tile([64, 512], FP32, tag="O",
                                              name=f"Ot{c//4}")
                psA = ps_ab.tile([128, KP], FP32, tag="A")
                psB = ps_ab.tile([128, KP], FP32, tag="B")
                nc.tensor.matmul(psA, t_qt2[0:64, sl], t_KRIT[0:64, :],
                                 start=True, stop=True, tile_position=(0, 0))
                nc.tensor.matmul(psB, t_qt2[64:128, sl], t_KRIT[64:128, :],
                                 start=True, stop=True, tile_position=(64, 0))
                T1 = work.tile([128, KP], FP32, tag="T1")
                T2 = work.tile([128, KP], FP32, tag="T2")
                nc.vector.tensor_tensor(T1, t_C[c][:, 0:KP], psA, mult)
                nc.vector.tensor_tensor(T2, t_S[c][:, 0:KP], psB, mult)
                L = work.tile([128, KP], FP32, tag="L")
                nc.gpsimd.tensor_tensor(L, T1, T2, sub)
                negmx = small.tile([128, 1], FP32, tag="negmx")
                nc.vector.tensor_reduce(negmx, L, mybir.AxisListType.X,
                                        mybir.AluOpType.max, negate=True)
                E = work.tile([128, KP], FP32, tag="E")
                rs = small.tile([128, 1], FP32, tag="rs")
                nc.scalar.activation(E, L, Exp, bias=negmx[:, 0:1], scale=1.0,
                                     accum_out=rs)
                rrs = small.tile([128, 1], FP32, tag="rrs")
                nc.vector.reciprocal(rrs, rs)
                G = gh.tile([128, KT], FP16, tag="G")
                H = gh.tile([128, KT], FP16, tag="H")
                if cfg["g_on_dve"]:
                    # G = (E*rrs)*C fused on DVE; H = P*S on Pool with P from DVE
                    nc.vector.scalar_tensor_tensor(G[:, 0:KP], E, rrs[:, 0:1],
                                                   t_C[c][:, 0:KP], mult, mult)
                    P = work.tile([128, KP], FP32, tag="P")
                    if cfg["p_on_dve"]:
                        nc.vector.tensor_scalar(P, E, rrs[:, 0:1], None, mult)
                    else:
                        nc.scalar.activation(P, E, Copy, scale=rrs[:, 0:1])
                    nc.gpsimd.tensor_tensor(H[:, 0:KP], P, t_S[c][:, 0:KP], mult)
                else:
                    P = work.tile([128, KP], FP32, tag="P")
                    if cfg["p_on_dve"]:
                        nc.vector.tensor_scalar(P, E, rrs[:, 0:1], None, mult)
                    else:
                        nc.scalar.activation(P, E, Copy, scale=rrs[:, 0:1])
                    nc.gpsimd.tensor_tensor(G[:, 0:KP], P, t_C[c][:, 0:KP], mult)
                    nc.gpsimd.tensor_tensor(H[:, 0:KP], P, t_S[c][:, 0:KP], mult)
                nc.vector.memset(G[:, KP:KT], 0.0)
                nc.vector.memset(H[:, KP:KT], 0.0)
                if cfg["final_late"]:
                    GT = ghT.tile([128, KT], FP16, tag=f"GT{c}", name=f"GTt{c}")
                    HT = ghT.tile([128, KT], FP16, tag=f"HT{c}", name=f"HTt{c}")
                else:
                    GT = ghT.tile([128, KT], FP16, tag="GT")
                    HT = ghT.tile([128, KT], FP16, tag="HT")
                use_pe = (cfg["tr_mode"] == "pe" or
                          (cfg["tr_mode"] == "hybrid" and c % 2 == 0))
                if use_pe:
                    for ti, (t_in, t_out, nm) in enumerate(((G, GT, "g"), (H, HT, "h"))):
                        pst = ps_ght.tile([128, KT], FP16, tag="ghtp",
                                          name=f"ghtp{nm}{c}")
                        for j in range(3):
                            jsl = slice(j * 128, (j + 1) * 128)
                            nc.tensor.transpose(pst[:, jsl], t_in[:, jsl], t_ident)
                        cp = cfg["ght_copy"]
                        eng = (nc.scalar if cp == "act" else
                               nc.vector if cp == "dve" else
                               (nc.scalar if ti == 0 else nc.vector))
                        if eng is nc.scalar:
                            nc.scalar.activation(t_out, pst, Copy)
                        else:
                            nc.vector.tensor_copy(t_out, pst)
                else:
                    for j in range(3):
                        jsl = slice(j * 128, (j + 1) * 128)
                        nc.sync.dma_start_transpose(GT[:, jsl], G[:, jsl])
                        nc.sync.dma_start_transpose(HT[:, jsl], H[:, jsl])
                if cfg["final_late"]:
                    GTs.append(GT); HTs.append(HT)
                else:
                    og = psO[c // 4][:, (c % 4) * 128:(c % 4 + 1) * 128]
                    for j in range(3):
                        jsl = slice(j * 128, (j + 1) * 128)
                        nc.tensor.matmul(og, t_VR[j], GT[:, jsl],
                                         start=(j == 0), stop=False)
                        nc.tensor.matmul(og, t_VI[j], HT[:, jsl],
                                         start=False, stop=(j == 2))
                if cfg["out_loop"] and not cfg["final_late"] and c % 4 == 3:
                    g = c // 4
                    t_og = work.tile([64, 512], FP32, tag="og", name=f"og{g}")
                    nc.scalar.activation(t_og, psO[g], Copy)
                    nc.sync.dma_start(out_t[:, g * 512:(g + 1) * 512], t_og)

                if cfg["final_late"] and c % 8 == 7:
                    # batched final matmuls over the last 8 chunks; keep each
                    # output slice's accumulation group contiguous
                    for cc in range(c - 7, c + 1):
                        og = psO[cc // 4][:, (cc % 4) * 128:(cc % 4 + 1) * 128]
                        for j in range(3):
                            jsl = slice(j * 128, (j + 1) * 128)
                            nc.tensor.matmul(og, t_VR[j], GTs[cc][:, jsl],
                                             start=(j == 0), stop=False)
                            nc.tensor.matmul(og, t_VI[j], HTs[cc][:, jsl],
                                             start=False, stop=(j == 2))
                    for g in (c // 4 - 1, c // 4):
                        t_og = work.tile([64, 512], FP32, tag="og", name=f"og{g}")
                        nc.scalar.activation(t_og, psO[g], Copy)
                        nc.sync.dma_start(out_t[:, g * 512:(g + 1) * 512], t_og)

            if not cfg["out_loop"]:
                for g in range(4):
                    t_og = work.tile([64, 512], FP32, tag="og", name=f"og{g}")
                    nc.scalar.activation(t_og, psO[g], Copy)
                    nc.sync.dma_start(out_t[:, g * 512:(g + 1) * 512], t_og)
            if ps_ght is not None:
                ps_ght.release()
            ps_out.release()
            ps_ab.release()
    return nc


_NC_CACHE = {}


def _get_nc(cfg=None):
    key = tuple(sorted((cfg or {}).items()))
    if key not in _NC_CACHE:
        _install_bir_patch()
        _NC_CACHE[key] = _build_nc(cfg)
    return _NC_CACHE[key]


# ------------------------------------------------------------- host wrapper
def _kvecs(nk):
    kx = np.arange(0, nk[0] + 1)
    ky = np.arange(-nk[1], nk[1] + 1)
    kz = np.arange(-nk[2], nk[2] + 1)
    KX, KY, KZ = np.meshgrid(kx, ky, kz, indexing="ij")
    return np.stack([KX, KY, KZ], axis=-1).reshape(-1, 3).astype(np.float64)


def _prep_core_inputs(q, k, v, r, box):
    n = N_NODE
    f = (r.astype(np.float64) / box[None, :]).astype(np.float32)  # [n,3] in [0,1)

    nk = [max(1, int(b)) for b in (box / DL).astype(np.int64)]
    kv = _kvecs(nk)
    ksq = TWOPI ** 2 * ((kv / box[None, :]) ** 2).sum(-1)
    valid = (ksq <= K_SQ_MAX) & (ksq > 0)
    kint = kv[valid].astype(np.float32)  # [Kv,3] small integers
    Kv = kint.shape[0]
    assert Kv <= 297, f"valid k-points {Kv} exceed padded width"

    fh = f.astype(ml_dtypes.bfloat16).astype(np.float32)
    # scale the residual by 2^14 (exact) so fp16 operands avoid subnormals;
    # the matching rhs rows carry kint * 2^-14.
    fl = ((f - fh) * 16384.0).astype(np.float16).astype(np.float32)

    kcolT = np.zeros((3, KP), np.float32)
    kcolT[:, :Kv] = kint.T

    lhsb = np.zeros((128, N_NODE), np.float32)
    lhsb[0:3] = fh.T
    lhsb[3] = 1.0
    lhsb[4] = 1.0
    lhsb[5:8] = fh.T
    lhsb[32:35] = fh.T
    lhsb[35] = 1.0
    lhsb[36] = 1.0
    lhsb[37] = 1.0
    lhsb[38:41] = fh.T
    lhsb[41] = 1.0

    rhsb = np.zeros((128, KP), np.float32)
    rhsb[0:3] = kcolT
    rhsb[3] = MAGIC
    rhsb[4] = -MAGIC
    rhsb[5:8] = -kcolT
    rhsb[32:35] = kcolT
    rhsb[35] = 0.25
    rhsb[36] = MAGIC
    rhsb[37] = -MAGIC
    rhsb[38:41] = -kcolT
    rhsb[41] = -0.25

    lhsh = np.zeros((128, N_NODE), np.float32)
    lhsh[0:3] = fl.T
    lhsh[32:35] = fl.T
    rhsh = np.zeros((128, KP), np.float32)
    rhsh[0:3] = -kcolT / 16384.0
    rhsh[32:35] = -kcolT / 16384.0

    kvc = np.concatenate([k, v], axis=1)  # [n,128]
    kvcat = np.ascontiguousarray(
        kvc.reshape(NCHUNK, 128, 128).transpose(1, 0, 2).reshape(128, NCHUNK * 128)
    ).astype(np.float32)
    qt = np.ascontiguousarray(q.T).astype(np.float32)
    qt2 = np.concatenate([qt, qt], axis=0)

    return {
        "lhsb": lhsb.astype(ml_dtypes.bfloat16),
        "lhsh": lhsh.astype(np.float16),
        "rhsb": rhsb.astype(ml_dtypes.bfloat16),
        "rhsh": rhsh.astype(np.float16),
        "kvcat": kvcat,
        "qt2": qt2,
    }


def kernel(q_vector, k_vector, v_vector, positions, cell, batch):
    q_vector = np.asarray(q_vector)
    k_vector = np.asarray(k_vector)
    v_vector = np.asarray(v_vector)
    positions = np.asarray(positions)
    cell = np.asarray(cell)

    n = N_NODE
    boxes = np.diagonal(cell.reshape(-1, 3, 3), axis1=-2, axis2=-1)  # [B,3]

    in_maps = []
    for b in range(B):
        sl = slice(b * n, (b + 1) * n)
        in_maps.append(_prep_core_inputs(
            q_vector[sl], k_vector[sl], v_vector[sl], positions[sl],
            boxes[b].astype(np.float64)))

    nc = _get_nc()
    res = None
    last_err = None
    for _attempt in range(3):
        try:
            res = run_bass_kernel_spmd(nc, in_maps, list(range(B)))
            break
        except Exception as e:  # transient device states (NRT unrecoverable)
            last_err = e
    if res is None:
        raise last_err

    out = np.empty((B * n, D), np.float32)
    for b in range(B):
        out[b * n:(b + 1) * n] = res.results[b]["out_t"].T
    return out


if __name__ == "__main__":
    rng = np.random.default_rng(0)
    inputs = {
        "q_vector": rng.standard_normal((B * N_NODE, D), dtype=np.float32),
        "k_vector": rng.standard_normal((B * N_NODE, D), dtype=np.float32),
        "v_vector": rng.standard_normal((B * N_NODE, D), dtype=np.float32),
        "positions": rng.uniform(0, 20, (B * N_NODE, 3)).astype(np.float32),
        "cell": np.tile((np.eye(3, dtype=np.float32) * 20.0)[None], (B, 1, 1)),
        "batch": np.repeat(np.arange(B, dtype=np.int32), N_NODE),
    }
    o = kernel(**inputs)
    print("kernel ran, out", o.shape, o.dtype, float(np.abs(o).max()))

